# revision 29
# baseline (speedup 1.0000x reference)
"""Trainium2 Bass kernel for the dual-stream transformer block
(nn_Block_87840671138274).

Sharding: 8 cores = 4 batches x 2 streams. Core i handles batch i//2,
stream i%2 (0=x, 1=y) and produces that stream's full output. Each core
redundantly computes the *other* stream's LN + K/V projections (~12%
extra FLOPs) so there are zero collectives.

Layout: everything transposed ([D, S] with D on SBUF partitions).
- Host pre-transposes inputs and pre-folds LN gamma/beta, softmax SCALE,
  the ws/wc stream-mix weights (into V and its bias), and a x16 fp8
  range scale (into V, un-done in Wo) -- all exact algebra in f32.
- Weights land in DRAM pre-tiled as [128, cols] so each matrix loads in
  a few big DMAs (2KB+ per partition line) through a rotating chunk
  pool; the rotation's WAR deps throttle the prefetch depth. W2 is
  re-prefetched for the second MLP half (re-reading wave-0 tiles would
  deadlock the rotation against the in-order PE queue).
- LN stats (mean / mean-of-squares) via ones-matmul partition
  reductions; squares on the DVE (bf16 2x mode) keeping ACT for exp.
- Attention per head pair (td) and type (self/cross): s^T = k^T.T @ q^T
  with q zero-padded per head to a full K=128 contraction. (K=64
  row-tiled pairs measure 2x faster in a dense microbench but starve
  the PE HAM activity monitor in the real ACT-bound phase -- it held
  the clock at 1.2 GHz for the whole 400us attention window.)
- exp on ACT writes fp8e4 directly, into jt-pair interleaved planes
  [128, 2, S]; the context matmul is then an fp8 DoubleRow matmul
  (contracts 2 jt blocks per instruction, 2x PE throughput) against
  v8 = [16*ws*v | ones] packed per head at stride 128. The ones column
  accumulates the softmax denominator in the same PSUM tile (row 64).
  Context matmuls are emitted one jt-pair late so the next scores
  (which feed ACT) keep queue priority on the PE.
- Softmax normalize: denominator row copied to SBUF with a native DVE
  op, then reciprocal_approx_fast (the custom-DVE op issues ~5x faster
  than the exact reciprocal but races in-flight PSUM accumulation if
  fed PSUM directly -- it reads a partial sum).
- No max-subtraction in softmax: scores are ~N(0, 0.31), |s| < ~2, and
  exp(s) fits fp8e4 comfortably.
- The MLP runs fully in fp8 DoubleRow as well: W1/W2 are host-scaled
  x32 into fp8 range (descaled for free via the gelu input-scale and
  one tensor_scalar at the output evac), xn2 and gelu outputs are
  written as fp8 contraction-pair planes. Total rel err 1.84e-2 vs the
  2e-2 gate, deterministic (fixed-seed inputs, deterministic engines).
- bf16 matmuls elsewhere with f32 PSUM accumulation; residual f32.
- LN2 statistics interleave with the out-projection: each x1 tile feeds
  the ones-matmul accumulators as soon as it exists, so the only LN2
  serial tail before the MLP is var->rstd->xn.
"""
import os
import numpy as np
import ml_dtypes

P = 128
S = 1024
D = 768
F = 3072
NH = 12
HD = 64
KT = D // P     # 6
JT = S // P     # 8
FT = F // P     # 24
EPS = 1e-6
SCALE = np.float32(1.0 / np.sqrt(HD))
BF = ml_dtypes.bfloat16
F8 = ml_dtypes.float8_e4m3
WCH = 2304      # weight chunk columns (3 x 768)

_PROGRAM = None


def _build_program():
    import concourse.bass as bass
    import concourse.bacc as bacc
    import concourse.tile as tile
    from concourse import mybir
    from contextlib import ExitStack

    f32 = mybir.dt.float32
    bf16 = mybir.dt.bfloat16
    fp8 = mybir.dt.float8e4
    DR = mybir.MatmulPerfMode.DoubleRow
    Ax = mybir.AluOpType
    Act = mybir.ActivationFunctionType

    nc = bacc.Bacc("TRN2", target_bir_lowering=False, debug=False, num_devices=8)

    aT_d = nc.dram_tensor("aT", [P, KT * S], f32, kind="ExternalInput").ap()
    aTb_d = nc.dram_tensor("aTb", [P, KT * S], bf16, kind="ExternalInput").ap()
    oT_d = nc.dram_tensor("oT", [P, KT * S], bf16, kind="ExternalInput").ap()
    w_d = {}
    for w in ("wq", "wk", "wv", "wk2", "wv2", "wo"):
        w_d[w] = nc.dram_tensor(w, [P, KT * D], bf16, kind="ExternalInput").ap()
    w_d["w1"] = nc.dram_tensor("w1", [P, KT * F], mybir.dt.float8e4,
                               kind="ExternalInput").ap()
    w_d["w2"] = nc.dram_tensor("w2", [P, FT * D], mybir.dt.float8e4,
                               kind="ExternalInput").ap()
    bpack_d = nc.dram_tensor("bpack", [P, 56], f32, kind="ExternalInput").ap()
    bvpack_d = nc.dram_tensor("bvpack", [1, 2 * D], bf16, kind="ExternalInput").ap()
    out_d = nc.dram_tensor("outT", [D, S], f32, kind="ExternalOutput").ap()
    dbg = {}
    if os.environ.get("KERNEL_DEBUG"):
        for nm in ("dbgq", "dbgk", "dbgp", "dbgc"):
            dbg[nm] = nc.dram_tensor(nm, [P, S], bf16, kind="ExternalOutput").ap()
        dbg["dbgv"] = nc.dram_tensor("dbgv", [P, 2 * NH * P], bf16,
                                     kind="ExternalOutput").ap()
        for nm in ("dbgden", "dbgrc", "dbgrb"):
            dbg[nm] = nc.dram_tensor(nm, [P, S], f32, kind="ExternalOutput").ap()

    NCHUNK = {"wq": 2, "wk": 2, "wk2": 2, "wv": 2, "wv2": 2, "wo": 2,
              "w1": 8, "w2": 8}

    with tile.TileContext(nc) as tc:
        with ExitStack() as ctx:
            perm = ctx.enter_context(tc.tile_pool(name="perm", bufs=1))
            res_pool = ctx.enter_context(tc.tile_pool(name="res", bufs=7))
            oT_pool = ctx.enter_context(tc.tile_pool(name="oTp", bufs=6))
            bb = ctx.enter_context(tc.tile_pool(name="bigbf", bufs=41 if dbg else 44))
            fw = ctx.enter_context(tc.tile_pool(name="f32w", bufs=5))
            wpool = ctx.enter_context(tc.tile_pool(name="wpool", bufs=6))

            wch = {}

            def prefetch_w(name, wave=0):
                if name == "w1":
                    # fp8 DR layout: [pr(3), pl(2), F]; chunk = (pr, F-half)
                    view = w_d["w1"].rearrange("p (a b c) -> p a b c",
                                               a=3, b=2)
                    for pr in range(3):
                        for hf in range(2):
                            t = wpool.tile([P, 2, F // 2], fp8,
                                           name=f"w1c{pr}_{hf}", tag="wc")
                            nc.sync.dma_start(
                                t[:], view[:, pr, :,
                                           hf * (F // 2):(hf + 1) * (F // 2)])
                            wch[("w1", pr * 2 + hf, wave)] = t
                    return
                if name == "w2":
                    # fp8 DR layout: [pr(12), pl(2), D]; chunk = 2 pairs
                    view = w_d["w2"].rearrange("p (a b c) -> p a b c",
                                               a=12, b=2)
                    for ci in range(6):
                        t = wpool.tile([P, 2, 2, D], fp8,
                                       name=f"w2c{ci}w{wave}", tag="wc")
                        nc.sync.dma_start(t[:], view[:, 2 * ci:2 * ci + 2,
                                                     :, :])
                        wch[("w2", ci, wave)] = t
                    return
                nch = NCHUNK[name]
                for c in range(nch):
                    t = wpool.tile([P, WCH], bf16, name=f"{name}c{c}w{wave}",
                                   tag="wc")
                    nc.sync.dma_start(t[:], w_d[name][:, c * WCH:(c + 1) * WCH])
                    wch[(name, c, wave)] = t

            def wslice(name, gcol, width, wave=0):
                """View of weight `name` columns [gcol, gcol+width)."""
                c, off = divmod(gcol, WCH)
                assert off + width <= WCH, (name, gcol, width)
                return wch[(name, c, wave)][:, off:off + width]

            def bbt(name, shape=(P, S)):
                return bb.tile(list(shape), bf16, name=name, tag="bb")

            def fwt(name, shape=(P, S)):
                return fw.tile(list(shape), f32, name=name, tag="fw")

            def act_recip(out_ap, in_ap):
                # Table-based reciprocal on the Scalar engine for LN rstd.
                se = nc.scalar
                se.add_instruction(mybir.InstActivation(
                    name=nc.get_next_instruction_name(),
                    func=Act.Reciprocal,
                    ins=[se.lower_ap(in_ap),
                         mybir.ImmediateValue(dtype=f32, value=0.0),
                         mybir.ImmediateValue(dtype=f32, value=1.0),
                         mybir.ImmediateValue(dtype=f32, value=0.0)],
                    outs=[se.lower_ap(out_ap)]))

            ones_kk = perm.tile([P, P], bf16, name="ones_kk")
            nc.gpsimd.memset(ones_kk[:], 1.0)
            ones_row = perm.tile([1, P], bf16, name="ones_row")
            nc.gpsimd.memset(ones_row[:], 1.0)
            eps_t = perm.tile([P, 1], f32, name="eps_t")
            nc.gpsimd.memset(eps_t[:], float(EPS))

            # ---- input + bias DMAs (emitted first so LN starts early) ----
            aTb = []
            for kt in range(KT):
                t = bbt(f"aTb{kt}")
                nc.sync.dma_start(t[:], aTb_d[:, kt * S:(kt + 1) * S])
                aTb.append(t)
            aT = []
            for kt in range(KT):
                t = res_pool.tile([P, S], f32, name=f"aT{kt}", tag="res")
                nc.sync.dma_start(t[:], aT_d[:, kt * S:(kt + 1) * S])
                aT.append(t)
            oT = []
            for kt in range(KT):
                t = oT_pool.tile([P, S], bf16, name=f"oT{kt}", tag="oT")
                nc.sync.dma_start(t[:], oT_d[:, kt * S:(kt + 1) * S])
                oT.append(t)

            bpack = perm.tile([P, 56], f32, name="bpack")
            nc.sync.dma_start(bpack[:], bpack_d[:])
            bq_t = bpack[:, 0:6]
            bk_t = bpack[:, 6:12]
            bk2_t = bpack[:, 12:18]
            bo_t = bpack[:, 18:24]
            b2_t = bpack[:, 24:30]
            b1_t = bpack[:, 30:54]
            bvpack = perm.tile([1, 2 * D], bf16, name="bvpack")
            nc.sync.dma_start(bvpack[:], bvpack_d[:])

            # ---- weight prefetch (q/k/k2/v/v2/o; rotation throttles) ----
            for nm in ("wq", "wk", "wk2", "wv", "wv2", "wo"):
                prefetch_w(nm)

            # ---- layer norm (transposed layout) ----
            def ln_T(src, src_bf, ln_psum, name):
                tbf, tsq = [], []
                for kt in range(KT):
                    if src_bf is not None:
                        c = src_bf[kt]
                    else:
                        c = bbt(f"{name}bf{kt}")
                        nc.vector.tensor_copy(c[:], src[kt][:])
                    q = bbt(f"{name}sq{kt}")
                    nc.vector.tensor_tensor(q[:], c[:], c[:], Ax.mult)
                    tbf.append(c)
                    tsq.append(q)
                msum = ln_psum.tile([P, S], f32, name=f"{name}ms", tag="lnms")
                sqsum = ln_psum.tile([P, S], f32, name=f"{name}vs", tag="lnvs")
                for kt in range(KT):
                    st, sp = kt == 0, kt == KT - 1
                    for nh in range(2):
                        sl = slice(nh * 512, (nh + 1) * 512)
                        nc.tensor.matmul(msum[:, sl], ones_kk[:], tbf[kt][:, sl],
                                         start=st, stop=sp)
                        nc.tensor.matmul(sqsum[:, sl], ones_kk[:], tsq[kt][:, sl],
                                         start=st, stop=sp)
                m_s = fwt(f"{name}m")
                nc.vector.tensor_scalar(m_s[:], msum[:], 1.0 / D, None, Ax.mult)
                cens = []
                for kt in range(KT):
                    cen = bbt(f"{name}cen{kt}")
                    nc.vector.scalar_tensor_tensor(cen[:], msum[:], -1.0 / D,
                                                   src[kt][:], Ax.mult, Ax.add)
                    cens.append(cen)
                m2 = fwt(f"{name}m2")
                nc.vector.tensor_tensor(m2[:], m_s[:], m_s[:], Ax.mult)
                var = fwt(f"{name}var")
                nc.vector.scalar_tensor_tensor(var[:], sqsum[:], 1.0 / D, m2[:],
                                               Ax.mult, Ax.subtract)
                std = fwt(f"{name}std")
                nc.scalar.activation(std[:], var[:], Act.Sqrt, bias=eps_t[:])
                rstd = bbt(f"{name}rstd")
                act_recip(rstd[:], std[:])
                xn = []
                for kt in range(KT):
                    x = bbt(f"{name}xn{kt}")
                    nc.vector.tensor_tensor(x[:], cens[kt][:], rstd[:], Ax.mult)
                    xn.append(x)
                return xn

            with nc.named_scope("ln1"):
                with tc.tile_pool(name="lnps_a", bufs=2, space="PSUM") as lnps:
                    xnA = ln_T(aT, aTb, lnps, "A")
                    xnO = ln_T(oT, oT, lnps, "O")

            # bias rows for natural-layout V, broadcast to 128 partitions
            # via a K=1 ones matmul
            bv_rows = []
            with tc.tile_pool(name="bvp", bufs=1, space="PSUM") as bvp:
                for nm in ("bv", "bv2"):
                    off = 0 if nm == "bv" else D
                    ps = bvp.tile([P, D], f32, name=f"{nm}ps", tag="bvps")
                    for sl in (slice(0, 512), slice(512, 768)):
                        nc.tensor.matmul(ps[:, sl], ones_row[:],
                                         bvpack[:, off + sl.start:off + sl.stop],
                                         start=True, stop=True)
                    bbx = perm.tile([P, D], bf16, name=f"{nm}bcast")
                    nc.vector.tensor_copy(bbx[:], ps[:])
                    bv_rows.append(bbx)
            bv_b, bv2_b = bv_rows

            # ---- projections ----
            # q/k stay packed [128, S] (two heads per tile); the score matmuls
            # contract K=64 via PE row tiling, pairing the two heads on the
            # two row-halves of the array (concurrent -> full throughput).
            qP = [bbt(f"qP{h}") for h in range(NH)]
            kTt = [bbt(f"kT{m}") for m in range(KT)]
            k2T = [bbt(f"k2T{m}") for m in range(KT)]
            # v in fp8, jt-pair plane-interleaved for DoubleRow context
            # matmuls: v8[jtp][:, par, h*128:h*128+64] = v rows of jt=2*jtp+par
            # (x16 scaled, ws/wc folded), col h*128+64 = softmax-denominator
            # ones, cols 65:128 zero (their cacc rows are ignored).
            v8A = [bb.tile([P, 2, NH * P], fp8, name=f"v8A{j}", tag="v8",
                           bufs=8) for j in range(JT // 2)]
            v8B = [bb.tile([P, 2, NH * P], fp8, name=f"v8B{j}", tag="v8",
                           bufs=8) for j in range(JT // 2)]

            with nc.named_scope("proj"):
                with tc.tile_pool(name="projps", bufs=3, space="PSUM") as pps:
                    # transposed-output projections: q, k, k2
                    for pname, xn, bias, dst in (("wq", xnA, bq_t, None),
                                                 ("wk", xnA, bk_t, kTt),
                                                 ("wk2", xnO, bk2_t, k2T)):
                        for mt in range(KT):
                            ps = pps.tile([P, S], f32, name=f"{pname}ps{mt}",
                                          tag="pps")
                            for kt in range(KT):
                                st, sp = kt == 0, kt == KT - 1
                                for nh in range(2):
                                    sl = slice(nh * 512, (nh + 1) * 512)
                                    nc.tensor.matmul(
                                        ps[:, sl],
                                        wslice(pname, kt * D + mt * P, P),
                                        xn[kt][:, sl], start=st, stop=sp)
                            if dst is not None:
                                nc.vector.tensor_scalar(dst[mt][:], ps[:],
                                                        bias[:, mt:mt + 1],
                                                        None, Ax.add)
                            else:
                                for hh in range(2):
                                    h, po = 2 * mt + hh, hh * 64
                                    t = qP[h]
                                    nc.gpsimd.memset(t[:], 0.0)
                                    nc.vector.tensor_scalar(
                                        t[po:po + 64, :], ps[po:po + 64, :],
                                        bias[po:po + 64, mt:mt + 1], None,
                                        Ax.add)
                    # natural-layout projections with ones column: v, v2
                    for pname, xn, bcast, dst in (("wv", xnA, bv_b, v8A),
                                                  ("wv2", xnO, bv2_b, v8B)):
                        for jt in range(JT):
                            ps = pps.tile([P, D], f32, name=f"{pname}ps{jt}",
                                          tag="pps")
                            for kt in range(KT):
                                st, sp = kt == 0, kt == KT - 1
                                for sl in (slice(0, 512), slice(512, 768)):
                                    nc.tensor.matmul(
                                        ps[:, sl],
                                        xn[kt][:, jt * P:(jt + 1) * P],
                                        wslice(pname, kt * D + sl.start,
                                               sl.stop - sl.start),
                                        start=st, stop=sp)
                            v8t = dst[jt // 2]
                            par = jt % 2
                            if par == 0:
                                nc.gpsimd.memset(v8t[:], 0.0)
                            dst_v = v8t[:, par:par + 1, :].rearrange(
                                "p one (h c) -> p (one h) c", c=P)
                            nc.vector.tensor_tensor(
                                dst_v[:, :, 0:64],
                                ps.rearrange("p (h c) -> p h c", c=64)[:],
                                bcast.rearrange("p (h c) -> p h c", c=64)[:],
                                Ax.add)
                            nc.gpsimd.memset(dst_v[:, :, 64:65], 1.0)

            # prefetch W1 (rotation makes these wait for dead proj chunks)
            prefetch_w("w1")

            if dbg:
                nc.sync.dma_start(dbg["dbgq"][:], qP[0][:])
                nc.sync.dma_start(dbg["dbgk"][:], kTt[0][:])
                dvt = bb.tile([P, 2 * NH * P], mybir.dt.bfloat16, name="dbgvt", tag="dbgv8", bufs=1)
                nc.vector.tensor_copy(
                    dvt.rearrange("p (a b) -> p a b", a=2)[:], v8A[0][:])
                nc.sync.dma_start(dbg["dbgv"][:], dvt[:])

            # ---- attention ----
            # Per (td, typ): the two heads of the tile pair run on the two
            # row-halves of the PE array (K=64 row tiling, concurrent).
            # exp writes fp8 into jt-pair planes; context matmuls are fp8
            # DoubleRow (contract 2 jt blocks per instruction).
            ctx_t = [bbt(f"ctx{t}") for t in range(KT)]
            with nc.named_scope("attn"):
                with tc.tile_pool(name="attnps", bufs=1, space="PSUM") as aps:
                    for td in range(KT):
                        tmps = [[None, None], [None, None]]  # [typ][hh]
                        for typ, (kk, v8) in enumerate(((kTt, v8A),
                                                        (k2T, v8B))):
                            caccs = []
                            for hh in range(2):
                                caccs.append(aps.tile(
                                    [P, S], f32, name=f"cv{td}_{typ}_{hh}",
                                    tag="cv", bufs=2))
                            def emit_ctx(jtp, pTpair):
                                for hh in range(2):
                                    h = 2 * td + hh
                                    for nh in range(2):
                                        sl = slice(nh * 512, (nh + 1) * 512)
                                        nc.tensor.matmul(
                                            caccs[hh][:, sl],
                                            v8[jtp][:, :, h * P:(h + 1) * P],
                                            pTpair[hh][:, :, sl],
                                            start=(jtp == 0),
                                            stop=(jtp == JT // 2 - 1),
                                            perf_mode=DR)

                            pTs = [None, None]
                            pend = None
                            for jt in range(JT):
                                jtp, par = jt // 2, jt % 2
                                sTs = []
                                for hh in range(2):
                                    sT = aps.tile(
                                        [P, S], f32,
                                        name=f"sT{td}_{typ}_{jt}_{hh}",
                                        tag="sT", bufs=2)
                                    for nh in range(2):
                                        sl = slice(nh * 512, (nh + 1) * 512)
                                        nc.tensor.matmul(
                                            sT[:, sl],
                                            kk[td][:, jt * P:(jt + 1) * P],
                                            qP[2 * td + hh][:, sl],
                                            start=True, stop=True)
                                    sTs.append(sT)
                                for hh in range(2):
                                    if par == 0:
                                        pTs[hh] = bb.tile(
                                            [P, 2, S], fp8,
                                            name=f"pT{td}_{typ}_{jtp}_{hh}",
                                            tag="bb")
                                    nc.scalar.activation(pTs[hh][:, par, :],
                                                         sTs[hh][:], Act.Exp)
                                if par == 1:
                                    # context is emitted one jt-pair late so
                                    # the next pair's score matmuls (which
                                    # feed ACT) take queue priority on PE
                                    if pend is not None:
                                        emit_ctx(*pend)
                                    pend = (jtp, list(pTs))
                                if dbg and td == 0 and typ == 0 and jt == 0:
                                    dpt = bbt("dbgpt")
                                    nc.vector.tensor_copy(
                                        dpt[:], pTs[0][:, 0, :])
                                    nc.sync.dma_start(dbg["dbgp"][:], dpt[:])
                            emit_ctx(*pend)
                            # normalize: ws/wc are folded into v host-side, so
                            # this is cacc[0:64] * (1/den), bank-aligned halves
                            for hh in range(2):
                                h = 2 * td + hh
                                cacc = caccs[hh]
                                # native-op copy to SBUF first: the custom DVE
                                # reciprocal races PSUM writes from the matmul
                                # accumulation group (reads a partial sum)
                                dcp = fwt(f"dc{h}_{typ}", (1, S))
                                nc.vector.tensor_copy(dcp[:], cacc[64:65, :])
                                recip = fwt(f"rc{h}_{typ}", (1, S))
                                nc.vector.reciprocal_approx_fast(recip[:],
                                                                 dcp[:])
                                rb = fwt(f"rb{h}_{typ}", (64, S))
                                nc.gpsimd.partition_broadcast(rb[:], recip[:])
                                tmp = bbt(f"tm{h}_{typ}", (64, S))
                                nc.vector.tensor_tensor(
                                    tmp[:], cacc[0:64, :], rb[:], Ax.mult)
                                tmps[typ][hh] = tmp
                                if dbg and h == 0 and typ == 0:
                                    dden = fwt("dbgdenT")
                                    nc.vector.tensor_copy(dden[0:1, :],
                                                          cacc[64:65, :])
                                    nc.sync.dma_start(dbg["dbgden"][0:1, :],
                                                      dden[0:1, :])
                                    nc.sync.dma_start(dbg["dbgrc"][0:1, :],
                                                      recip[:])
                                    nc.sync.dma_start(dbg["dbgrb"][0:64, :],
                                                      rb[:])
                        for hh in range(2):
                            po = hh * 64
                            nc.vector.tensor_add(ctx_t[td][po:po + 64, :],
                                                 tmps[0][hh][:],
                                                 tmps[1][hh][:])
                        if dbg and td == 0:
                            nc.sync.dma_start(dbg["dbgc"][:], ctx_t[0][:])

            # prefetch W2
            prefetch_w("w2")

            # ---- out-projection + residual, LN2 stats interleaved ----
            x1 = []
            x1b = []
            with nc.named_scope("outproj"):
                with tc.tile_pool(name="opps", bufs=2, space="PSUM") as ops, \
                     tc.tile_pool(name="lnps_b", bufs=1, space="PSUM") as lnps2:
                    msumB = lnps2.tile([P, S], f32, name="Bms", tag="lnms")
                    sqsumB = lnps2.tile([P, S], f32, name="Bvs", tag="lnvs")
                    for mt in range(KT):
                        ps = ops.tile([P, S], f32, name=f"ops{mt}", tag="ops")
                        for kt in range(KT):
                            st, sp = kt == 0, kt == KT - 1
                            for nh in range(2):
                                sl = slice(nh * 512, (nh + 1) * 512)
                                nc.tensor.matmul(
                                    ps[:, sl],
                                    wslice("wo", kt * D + mt * P, P),
                                    ctx_t[kt][:, sl], start=st, stop=sp)
                        t = res_pool.tile([P, S], f32, name=f"x1_{mt}", tag="res")
                        nc.vector.scalar_tensor_tensor(t[:], ps[:],
                                                       bo_t[:, mt:mt + 1],
                                                       aT[mt][:], Ax.add, Ax.add)
                        x1.append(t)
                        c = bbt(f"x1b{mt}")
                        nc.vector.tensor_copy(c[:], t[:])
                        x1b.append(c)
                        q = bbt(f"x1sq{mt}")
                        nc.vector.tensor_tensor(q[:], c[:], c[:], Ax.mult)
                        st, sp = mt == 0, mt == KT - 1
                        for nh in range(2):
                            sl = slice(nh * 512, (nh + 1) * 512)
                            nc.tensor.matmul(msumB[:, sl], ones_kk[:],
                                             c[:, sl], start=st, stop=sp)
                            nc.tensor.matmul(sqsumB[:, sl], ones_kk[:],
                                             q[:, sl], start=st, stop=sp)
                    # LN2 tail: var -> rstd -> xn
                    m_s = fwt("Bm")
                    nc.vector.tensor_scalar(m_s[:], msumB[:], 1.0 / D, None,
                                            Ax.mult)
                    censB = []
                    for kt in range(KT):
                        cen = bbt(f"Bcen{kt}")
                        nc.vector.scalar_tensor_tensor(cen[:], msumB[:],
                                                       -1.0 / D, x1[kt][:],
                                                       Ax.mult, Ax.add)
                        censB.append(cen)
                    m2 = fwt("Bm2")
                    nc.vector.tensor_tensor(m2[:], m_s[:], m_s[:], Ax.mult)
                    var = fwt("Bvar")
                    nc.vector.scalar_tensor_tensor(var[:], sqsumB[:], 1.0 / D,
                                                   m2[:], Ax.mult, Ax.subtract)
                    std = fwt("Bstd")
                    nc.scalar.activation(std[:], var[:], Act.Sqrt, bias=eps_t[:])
                    rstd = bbt("Brstd")
                    act_recip(rstd[:], std[:])
                    xn2_8 = [bb.tile([P, 2, S], fp8, name=f"Bxn8{i}",
                                     tag="bb") for i in range(3)]
                    for kt in range(KT):
                        nc.vector.tensor_tensor(
                            xn2_8[kt // 2][:, kt % 2, :], censB[kt][:],
                            rstd[:], Ax.mult)

            # ---- MLP (fp8 DoubleRow, weights x32, descaled at evac) ----
            with nc.named_scope("mlp"):
                with tc.tile_pool(name="mlpps", bufs=3, space="PSUM") as mps:
                    h8 = [bb.tile([P, 2, S], fp8, name=f"h8_{i}", tag="bb")
                          for i in range(FT // 2)]
                    for fb in range(FT):
                        ps = mps.tile([P, S], f32, name=f"h_ps{fb}", tag="mps")
                        for pr in range(3):
                            st, sp = pr == 0, pr == 2
                            ch = wch[("w1", pr * 2 + fb // 12, 0)]
                            co = (fb % 12) * P
                            for nh in range(2):
                                sl = slice(nh * 512, (nh + 1) * 512)
                                nc.tensor.matmul(
                                    ps[:, sl], ch[:, :, co:co + P],
                                    xn2_8[pr][:, :, sl], start=st, stop=sp,
                                    perf_mode=DR)
                        nc.scalar.activation(h8[fb // 2][:, fb % 2, :], ps[:],
                                             Act.Gelu_apprx_tanh,
                                             bias=b1_t[:, fb:fb + 1],
                                             scale=1.0 / 32)
                    for half in range(2):
                        # each half re-reads every w2 chunk, so half 1 gets a
                        # fresh prefetch wave (slot rotation would deadlock on
                        # re-reads of wave-0 tiles behind it in the PE queue)
                        if half == 1:
                            prefetch_w("w2", wave=1)
                        psl = []
                        for ml in range(3):
                            ps = mps.tile([P, S], f32, name=f"o_ps{half}_{ml}",
                                          tag="mps")
                            psl.append(ps)
                        for pr in range(FT // 2):
                            ch = wch[("w2", pr // 2, half)]
                            sub = pr % 2
                            for ml in range(3):
                                mt = half * 3 + ml
                                st, sp = pr == 0, pr == FT // 2 - 1
                                for nh in range(2):
                                    sl = slice(nh * 512, (nh + 1) * 512)
                                    nc.tensor.matmul(
                                        psl[ml][:, sl],
                                        ch[:, sub, :, mt * P:(mt + 1) * P],
                                        h8[pr][:, :, sl], start=st, stop=sp,
                                        perf_mode=DR)
                        for ml in range(3):
                            mt = half * 3 + ml
                            t0 = fwt(f"o32_{mt}")
                            nc.vector.tensor_scalar(t0[:], psl[ml][:],
                                                    1.0 / 32,
                                                    b2_t[:, mt:mt + 1],
                                                    Ax.mult, Ax.add)
                            ot = fwt(f"out{mt}")
                            nc.vector.tensor_tensor(ot[:], t0[:], x1[mt][:],
                                                    Ax.add)
                            nc.sync.dma_start(out_d[mt * P:(mt + 1) * P, :],
                                              ot[:])

    nc.compile()
    return nc


def _get_program():
    global _PROGRAM
    if _PROGRAM is None:
        _PROGRAM = _build_program()
    return _PROGRAM


def _to128(W):
    """[768, C] -> [128, 6*C]: row-tile kt lands at columns [kt*C, (kt+1)*C)."""
    C = W.shape[1]
    return np.ascontiguousarray(
        W.reshape(KT, P, C).transpose(1, 0, 2).reshape(P, KT * C))


def _fold_core(inp, b, s):
    """Host-side shard + weight folding for core (batch b, stream s)."""
    if s == 0:
        a, o = inp['x'][b], inp['y'][b]
        g1s, b1s, g1o, b1o = inp['ln1x_g'], inp['ln1x_b'], inp['ln1y_g'], inp['ln1y_b']
        Wq, bq, Wk, bk, Wv, bv = inp['Wq'], inp['bq'], inp['Wk'], inp['bk'], inp['Wv'], inp['bv']
        Wk2, bk2, Wv2, bv2 = inp['Wkd'], inp['bkd'], inp['Wvd'], inp['bvd']
        Wo, bo = inp['Wo'], inp['bo']
        ws, wc = inp['w11'][0], inp['w12'][0]
        g2, b2g = inp['ln2x_g'], inp['ln2x_b']
        W1, b1, W2, b2 = inp['W1'], inp['b1'], inp['W2'], inp['b2']
    else:
        a, o = inp['y'][b], inp['x'][b]
        g1s, b1s, g1o, b1o = inp['ln1y_g'], inp['ln1y_b'], inp['ln1x_g'], inp['ln1x_b']
        Wq, bq, Wk, bk, Wv, bv = inp['Wqd'], inp['bqd'], inp['Wkd'], inp['bkd'], inp['Wvd'], inp['bvd']
        Wk2, bk2, Wv2, bv2 = inp['Wk'], inp['bk'], inp['Wv'], inp['bv']
        Wo, bo = inp['Wod'], inp['bod']
        ws, wc = inp['w21'][0], inp['w22'][0]
        g2, b2g = inp['ln2y_g'], inp['ln2y_b']
        W1, b1, W2, b2 = inp['W1d'], inp['b1d'], inp['W2d'], inp['b2d']

    aTf = _to128(np.ascontiguousarray(a.T, np.float32).reshape(D, S))
    oTf = _to128(np.ascontiguousarray(o.T, np.float32).reshape(D, S))

    # w1/w2: fp8 DoubleRow pair-interleaved, x32 range scale
    W1g = (32.0 * g2[:, None] * W1).astype(np.float32)   # [768, 3072]
    w1h = W1g.reshape(3, 2, P, F).transpose(2, 0, 1, 3).reshape(P, KT * F)
    W2f = (32.0 * np.asarray(W2, np.float32))             # [3072, 768]
    w2h = W2f.reshape(12, 2, P, D).transpose(2, 0, 1, 3).reshape(P, FT * D)

    bqv = (SCALE * (bq + b1s @ Wq)).astype(np.float32)
    bkv = (bk + b1s @ Wk).astype(np.float32)
    bk2v = (bk2 + b1o @ Wk2).astype(np.float32)
    bov = ((ws + wc) * bo).astype(np.float32)
    b2v = np.asarray(b2, np.float32)
    b1v = (b1 + b2g @ W1).astype(np.float32)
    bpack = np.zeros((P, 56), np.float32)
    bpack[:, 0:6] = bqv.reshape(KT, P).T
    bpack[:, 6:12] = bkv.reshape(KT, P).T
    bpack[:, 12:18] = bk2v.reshape(KT, P).T
    bpack[:, 18:24] = bov.reshape(KT, P).T
    bpack[:, 24:30] = b2v.reshape(KT, P).T
    bpack[:, 30:54] = b1v.reshape(FT, P).T
    bvrow = (16.0 * ws * (bv + b1s @ Wv)).astype(np.float32)
    bv2row = (16.0 * wc * (bv2 + b1o @ Wv2)).astype(np.float32)
    bvpack = np.concatenate([bvrow, bv2row]).reshape(1, 2 * D)

    m = {
        'aT': aTf,
        'aTb': aTf.astype(BF),
        'oT': oTf.astype(BF),
        'wq': _to128(g1s[:, None] * Wq * SCALE).astype(BF),
        'wk': _to128(g1s[:, None] * Wk).astype(BF),
        'wv': _to128(16.0 * ws * g1s[:, None] * Wv).astype(BF),
        'wk2': _to128(g1o[:, None] * Wk2).astype(BF),
        'wv2': _to128(16.0 * wc * g1o[:, None] * Wv2).astype(BF),
        'wo': _to128(np.asarray(Wo, np.float32) / 16.0).astype(BF),
        'w1': np.ascontiguousarray(w1h).astype(F8),
        'w2': np.ascontiguousarray(w2h).astype(F8),
        'bpack': bpack,
        'bvpack': bvpack.astype(BF),
    }
    return m


LAST_RESULTS = None


def kernel(**inputs):
    from concourse.bass_utils import run_bass_kernel_spmd
    global LAST_RESULTS

    inp = {k: np.asarray(v, np.float32) for k, v in inputs.items()}
    B = inp['x'].shape[0]

    nc = _get_program()
    in_maps = [_fold_core(inp, core // 2, core % 2) for core in range(2 * B)]
    tdir = os.environ.get("KERNEL_TRACE_DIR")
    if tdir:
        os.makedirs(tdir, exist_ok=True)
    res = run_bass_kernel_spmd(
        nc, in_maps, core_ids=list(range(2 * B)),
        trace=bool(os.environ.get("KERNEL_TRACE")),
        tmpdir=tdir)
    LAST_RESULTS = res

    x_out = np.empty((B, S, D), np.float32)
    y_out = np.empty((B, S, D), np.float32)
    for b in range(B):
        x_out[b] = res.results[2 * b]["outT"].T
        y_out[b] = res.results[2 * b + 1]["outT"].T
    return (x_out, y_out)
